# revision 7
# baseline (speedup 1.0000x reference)
"""Trainium2 Bass kernel for nn_BigraphModel (gnn_message_passing).

Strategy (8 NeuronCores, SPMD single NEFF):
  - Nodes are sharded into 8 equal contiguous ranges (12500 real + 44 pad rows
    per core so AllGather chunks are uniform 12544-row slices).
  - Edges are sharded by destination: every edge lands on the core that owns
    its dst node, so segment sums complete locally (no all-reduce).
  - Per layer, each core computes updated features for its owned nodes only;
    an AllGather replicates the per-layer gather table [100352, 128] to all
    cores. Layer 1 needs no AllGather (the full input x is already available).
  - Edge phase: big indirect-DMA row gathers (src/dst feature rows), per-edge
    cosine terms on DVE/ACT, and an in-tile segment-sum via a one-hot
    selection matmul on the PE (host precomputes per-edge slot ids; runs of a
    given dst never straddle a tile). Results stream to DRAM; the node phase
    gathers one stream row per owned node.
  - The linear layer W is applied after aggregation (linearity), so gather
    tables stay 128 channels wide.

Host-side numpy does only sharding/index prep: edge bucketing+sorting, slot
assignment, degree counts, padding, and final output reassembly.
"""

import os
import sys

import numpy as np

N, D, E, NCORES = 100000, 128, 600000, 8
SLICE_R = N // NCORES            # 12500 real nodes per core
SLICE_P = 12544                  # padded to multiple of 128
NPAD = SLICE_P * NCORES          # 100352 table rows
TILE_E = 128                     # edges per tile
TILE_S = 32                      # max slots (distinct dst) per tile
BLK = 4                          # tiles per superblock (4*32 = 128 psum slots)
NODE_BLK = 7                    # node tiles gathered per stream-gather call
NTILE_OWN = SLICE_P // 128       # 98
EPS = 1e-8

LAST_EXEC_NS = None


def _row_of_node(n):
    """Map node id -> padded table row."""
    return (n // SLICE_R) * SLICE_P + (n % SLICE_R)


def _prep_graph(src, dst, attr, dst_keep_mask, src_mask, split_by_src_mask):
    """Shard a graph's edges by dst owner; per core build tile/slot arrays.

    Returns (per_core list of dicts, NB) where every core has identical NB
    (superblock count), padded as needed.
    """
    cores = []
    owner = dst // SLICE_R
    cnt_all = np.bincount(dst, minlength=N)  # full in-degree (pre-filter)
    for c in range(NCORES):
        sel = owner == c
        if dst_keep_mask is not None:
            sel &= dst_keep_mask[dst]
        es, ed, ea = src[sel], dst[sel], attr[sel]
        eid = np.nonzero(sel)[0]
        order = np.argsort(ed, kind="stable")
        es, ed, ea, eid = es[order], ed[order], ea[order], eid[order]
        # run boundaries (consecutive equal dst)
        if len(ed):
            bnd = np.nonzero(np.diff(ed))[0] + 1
            starts = np.concatenate(([0], bnd))
            ends = np.concatenate((bnd, [len(ed)]))
        else:
            starts = ends = np.zeros(0, np.int64)
        run_len = ends - starts
        if len(run_len) and run_len.max() > TILE_E:
            raise ValueError("in-degree > 128 unsupported by this kernel")
        # greedy tile packing: <=128 edges, <=32 runs per tile
        tiles = []  # list of list of run indices
        cur, ce, cr = [], 0, 0
        for r in range(len(starts)):
            L = int(run_len[r])
            if ce + L > TILE_E or cr + 1 > TILE_S:
                tiles.append(cur)
                cur, ce, cr = [], 0, 0
            cur.append(r)
            ce += L
            cr += 1
        if cur:
            tiles.append(cur)
        cores.append(
            dict(es=es, ed=ed, ea=ea, eid=eid, starts=starts, ends=ends,
                 tiles=tiles, cnt=cnt_all)
        )
    nt_max = max(len(c["tiles"]) for c in cores)
    nb = max(1, -(-nt_max // BLK))
    nt_pad = nb * BLK
    out = []
    for c in range(NCORES):
        g = cores[c]
        tiles = g["tiles"]
        idx8 = np.zeros((nt_pad, TILE_E, 2), np.int32)      # [t,p,(src,dst)]
        attr_a = np.zeros((nt_pad, TILE_E), np.float32)
        sid_m = np.full((nt_pad, TILE_E), -1.0, np.float32)
        sid_u = np.full((nt_pad, TILE_E), -1.0, np.float32)
        rcnt = np.zeros((nt_pad, TILE_S), np.float32)
        pos = np.full(SLICE_P, nt_pad * TILE_S, np.int64)   # zero-row default
        orig = np.full((nt_pad, TILE_E), -1, np.int64)
        for t, runs in enumerate(tiles):
            p = 0
            for s, r in enumerate(runs):
                a, b = int(g["starts"][r]), int(g["ends"][r])
                L = b - a
                d_node = int(g["ed"][a])
                bias = (t % BLK) * TILE_S
                idx8[t, p:p + L, 0] = _row_of_node(g["es"][a:b])
                idx8[t, p:p + L, 1] = _row_of_node(np.int64(d_node))
                attr_a[t, p:p + L] = g["ea"][a:b]
                if split_by_src_mask is not None:
                    sm = split_by_src_mask[g["es"][a:b]]
                    sid_m[t, p:p + L] = np.where(sm, float(s + bias), -1.0)
                    sid_u[t, p:p + L] = np.where(sm, -1.0, float(s + bias))
                else:
                    sid_m[t, p:p + L] = float(s + bias)
                rcnt[t, s] = 1.0 / max(int(g["cnt"][d_node]), 1)
                # stream row for this dst: block*128 + (t%4)*32 + s
                pos[d_node % SLICE_R] = (t // BLK) * 128 + bias + s
                orig[t, p:p + L] = g["eid"][a:b]
                p += L
        # reshape into superblock layout [NB, 128, cols]
        i8 = idx8.reshape(nb, BLK, TILE_E, 2)
        idx = np.zeros((nb, TILE_E, 8), np.int32)
        for j in range(BLK):
            idx[:, :, j] = i8[:, j, :, 0]
            idx[:, :, 4 + j] = i8[:, j, :, 1]
        prm = np.zeros((nb, TILE_E, 13), np.float32)
        at = attr_a.reshape(nb, BLK, TILE_E)
        sm_ = sid_m.reshape(nb, BLK, TILE_E)
        su_ = sid_u.reshape(nb, BLK, TILE_E)
        for j in range(BLK):
            prm[:, :, j] = at[:, j]
            prm[:, :, 4 + j] = sm_[:, j]
            prm[:, :, 8 + j] = su_[:, j]
        prm[:, :, 12] = rcnt.reshape(nb, 128)
        posall = pos.reshape(NTILE_OWN, 128).T.astype(np.int32)  # [128, 98]
        orig_b = np.zeros((nb, TILE_E, BLK), np.int64)
        ob = orig.reshape(nb, BLK, TILE_E)
        for j in range(BLK):
            orig_b[:, :, j] = ob[:, j]
        out.append(dict(idx=idx, prm=prm, posall=posall, orig=orig_b))
    return out, nb


def _build(NBii, NBuu):
    import concourse.bass as bass
    import concourse.mybir as mybir
    import concourse.tile as tile
    from concourse import library_config
    from concourse.masks import make_identity

    f32 = mybir.dt.float32
    i32 = mybir.dt.int32
    AF = mybir.ActivationFunctionType
    ALU = mybir.AluOpType

    nc = bass.Bass()

    # ---- external inputs -------------------------------------------------
    x_full = nc.dram_tensor("x_full", [NPAD, D], f32, kind="ExternalInput")
    x_own = nc.dram_tensor("x_own", [SLICE_P, D], f32, kind="ExternalInput")
    w1t = nc.dram_tensor("w1t", [D, D], f32, kind="ExternalInput")
    w2t = nc.dram_tensor("w2t", [D, D], f32, kind="ExternalInput")
    wut = nc.dram_tensor("wut", [D, D], f32, kind="ExternalInput")
    b1 = nc.dram_tensor("b1", [D, D], f32, kind="ExternalInput")
    b2 = nc.dram_tensor("b2", [D, D], f32, kind="ExternalInput")
    bu = nc.dram_tensor("bu", [D, D], f32, kind="ExternalInput")
    iota4 = nc.dram_tensor("iota4", [D, 512], f32, kind="ExternalInput")
    maskt = nc.dram_tensor("maskt", [D, NTILE_OWN], mybir.dt.int8, kind="ExternalInput")
    idx_ii = nc.dram_tensor("idx_ii", [NBii, TILE_E, 8], i32, kind="ExternalInput")
    prm_ii = nc.dram_tensor("prm_ii", [NBii, TILE_E, 13], f32, kind="ExternalInput")
    pos_ii = nc.dram_tensor("pos_ii", [D, NTILE_OWN], i32, kind="ExternalInput")
    idx_uu = nc.dram_tensor("idx_uu", [NBuu, TILE_E, 8], i32, kind="ExternalInput")
    prm_uu = nc.dram_tensor("prm_uu", [NBuu, TILE_E, 13], f32, kind="ExternalInput")
    pos_uu = nc.dram_tensor("pos_uu", [D, NTILE_OWN], i32, kind="ExternalInput")
    cosout = nc.dram_tensor("cosout", [NBuu, TILE_E, 4], f32, kind="ExternalOutput")

    NSii = NBii * 128 + 128   # stream rows (+128 pad incl. zero row)
    NSuu = NBuu * 128 + 128
    ZRii = NBii * 128
    ZRuu = NBuu * 128

    with tile.TileContext(nc) as tc:
        with (
            tc.tile_pool(name="dram", bufs=1, space="DRAM") as dram,
            tc.tile_pool(name="const", bufs=1) as constp,
            tc.tile_pool(name="eg", bufs=3) as egp,
            tc.tile_pool(name="esm", bufs=3) as esmp,
            tc.tile_pool(name="ework", bufs=3) as ewp,
            tc.tile_pool(name="npool", bufs=3) as npp,
            tc.tile_pool(name="psum", bufs=2, space="PSUM") as psp,
            tc.tile_pool(name="psum2", bufs=2, space="PSUM") as psp2,
        ):
            # DRAM intermediates
            stream_i1 = dram.tile([NSii, 256], f32, tag="st_i1")
            stream_i2 = dram.tile([NSii, 256], f32, tag="st_i2")
            stream_u3 = dram.tile([NSuu, 128], f32, tag="st_u3")
            stream_u4 = dram.tile([NSuu, 128], f32, tag="st_u4")
            agin = [dram.tile([SLICE_P, D], f32, tag=f"agin{k}", name=f"agin{k}") for k in range(4)]
            tbl = [dram.tile([NPAD, D], f32, tag=f"tbl{k}", name=f"tbl{k}") for k in range(4)]

            # constants
            ident = constp.tile([D, D], f32, tag="ident")
            make_identity(nc, ident[:])
            iot = constp.tile([D, 512], f32, tag="iot")
            nc.sync.dma_start(out=iot[:], in_=iota4[:])
            wts = {}
            for nm, t in (("w1", w1t), ("w2", w2t), ("wu", wut),
                          ("b1", b1), ("b2", b2), ("bu", bu)):
                wt = constp.tile([D, D], f32, tag=f"c_{nm}", name=f"c_{nm}")
                nc.sync.dma_start(out=wt[:], in_=t[:])
                wts[nm] = wt
            maskc = constp.tile([D, NTILE_OWN], mybir.dt.int8, tag="maskc")
            nc.sync.dma_start(out=maskc[:], in_=maskt[:])
            posc_ii = constp.tile([D, NTILE_OWN], i32, tag="posc_ii")
            nc.sync.dma_start(out=posc_ii[:], in_=pos_ii[:])
            posc_uu = constp.tile([D, NTILE_OWN], i32, tag="posc_uu")
            nc.sync.dma_start(out=posc_uu[:], in_=pos_uu[:])
            zrow = constp.tile([D, 256], f32, tag="zrow")
            nc.vector.memset(zrow[:], 0.0)
            # zero the pad tail of every stream (gathered rows must be finite)
            for st, zr in ((stream_i1, ZRii), (stream_i2, ZRii)):
                nc.sync.dma_start(out=st[zr:zr + 128, :], in_=zrow[:, :256])
            for st, zr in ((stream_u3, ZRuu), (stream_u4, ZRuu)):
                nc.sync.dma_start(out=st[zr:zr + 128, :], in_=zrow[:, :128])


            # ---------------- edge phase helpers --------------------------
            def edge_phase_ea(table_ap, idx_t, prm_t, nb, stream_t):
                for b in range(nb):
                    idxt = esmp.tile([TILE_E, 8], i32, tag="e_idx")
                    nc.sync.dma_start(out=idxt[:], in_=idx_t[b])
                    prm = esmp.tile([TILE_E, 13], f32, tag="e_prm")
                    nc.sync.dma_start(out=prm[:], in_=prm_t[b])
                    g = egp.tile([TILE_E, 8 * D], f32, tag="e_g")
                    nc.gpsimd.indirect_dma_start(
                        out=g[:], out_offset=None, in_=table_ap,
                        in_offset=bass.IndirectOffsetOnAxis(ap=idxt[:], axis=0),
                    )
                    gs = g[:, 0:512].rearrange("p (j c) -> p j c", c=D)
                    gd = g[:, 512:1024].rearrange("p (j c) -> p j c", c=D)
                    # per-edge dot(x_s, x_d)
                    tmp = ewp.tile([TILE_E, 512], f32, tag="e_tmp")
                    nc.vector.tensor_tensor(
                        out=tmp[:], in0=g[:, 0:512], in1=g[:, 512:1024],
                        op=ALU.mult)
                    dotp = ewp.tile([TILE_E, 4], f32, tag="e_dot")
                    nc.vector.reduce_sum(
                        out=dotp[:], in_=tmp[:].rearrange("p (j c) -> p j c", c=D),
                        axis=mybir.AxisListType.X)
                    # per-edge ||x_s||^2 via ACT square+accum
                    ssq = ewp.tile([TILE_E, 4], f32, tag="e_ssq")
                    dump = ewp.tile([TILE_E, D], f32, tag="e_dump")
                    for j in range(4):
                        nc.scalar.activation(
                            out=dump[:], in_=gs[:, j, :], func=AF.Square,
                            accum_out=ssq[:, j:j + 1])
                    nrm = ewp.tile([TILE_E, 4], f32, tag="e_nrm")
                    nc.scalar.activation(out=nrm[:], in_=ssq[:], func=AF.Sqrt)
                    nc.vector.tensor_scalar(
                        out=nrm[:], in0=nrm[:], scalar1=EPS, scalar2=None,
                        op0=ALU.max)
                    nc.vector.reciprocal(out=nrm[:], in_=nrm[:])
                    beta = ewp.tile([TILE_E, 4], f32, tag="e_beta")
                    nc.vector.tensor_tensor(
                        out=beta[:], in0=dotp[:], in1=prm[:, 0:4], op=ALU.mult)
                    nc.vector.tensor_tensor(
                        out=beta[:], in0=beta[:], in1=nrm[:], op=ALU.mult)
                    # messages beta * x_s
                    mvg = ewp.tile([TILE_E, 512], f32, tag="e_mvg")
                    nc.vector.tensor_tensor(
                        out=mvg[:].rearrange("p (j c) -> p j c", c=D),
                        in0=gs, in1=beta[:].to_broadcast([TILE_E, 4, D]),
                        op=ALU.mult)
                    # selection matrices (masked / unmasked src)
                    stm = ewp.tile([TILE_E, 512], f32, tag="e_stm")
                    nc.vector.tensor_tensor(
                        out=stm[:].rearrange("p (j c) -> p j c", c=D),
                        in0=iot[:].rearrange("p (j c) -> p j c", c=D),
                        in1=prm[:, 4:8].to_broadcast([TILE_E, 4, D]),
                        op=ALU.is_equal)
                    stu = ewp.tile([TILE_E, 512], f32, tag="e_stu")
                    nc.vector.tensor_tensor(
                        out=stu[:].rearrange("p (j c) -> p j c", c=D),
                        in0=iot[:].rearrange("p (j c) -> p j c", c=D),
                        in1=prm[:, 8:12].to_broadcast([TILE_E, 4, D]),
                        op=ALU.is_equal)
                    psA = psp.tile([D, D], f32, tag="ps1")
                    psB = psp2.tile([D, D], f32, tag="ps2")
                    for j in range(4):
                        nc.tensor.matmul(
                            out=psA[:], lhsT=stm[:, j * D:(j + 1) * D],
                            rhs=mvg[:, j * D:(j + 1) * D],
                            start=(j == 0), stop=(j == 3))
                    for j in range(4):
                        nc.tensor.matmul(
                            out=psB[:], lhsT=stu[:, j * D:(j + 1) * D],
                            rhs=mvg[:, j * D:(j + 1) * D],
                            start=(j == 0), stop=(j == 3))
                    sA = egp.tile([TILE_E, 256], f32, tag="e_sA")
                    nc.vector.tensor_scalar(
                        out=sA[:, 0:D], in0=psA[:], scalar1=prm[:, 12:13],
                        scalar2=None, op0=ALU.mult)
                    nc.vector.tensor_scalar(
                        out=sA[:, D:256], in0=psB[:], scalar1=prm[:, 12:13],
                        scalar2=None, op0=ALU.mult)
                    nc.sync.dma_start(
                        out=stream_t[b * 128:(b + 1) * 128, :], in_=sA[:])

            def edge_phase_uiu(table_ap, idx_t, prm_t, nb, stream_t):
                for b in range(nb):
                    idxt = esmp.tile([TILE_E, 4], i32, tag="e_idx4")
                    nc.sync.dma_start(out=idxt[:], in_=idx_t[b, :, 0:4])
                    prm = esmp.tile([TILE_E, 13], f32, tag="e_prm")
                    nc.sync.dma_start(out=prm[:], in_=prm_t[b])
                    g = egp.tile([TILE_E, 4 * D], f32, tag="e_g4")
                    nc.gpsimd.indirect_dma_start(
                        out=g[:], out_offset=None, in_=table_ap,
                        in_offset=bass.IndirectOffsetOnAxis(ap=idxt[:], axis=0),
                    )
                    mvg = ewp.tile([TILE_E, 512], f32, tag="e_mvg")
                    nc.vector.tensor_tensor(
                        out=mvg[:].rearrange("p (j c) -> p j c", c=D),
                        in0=g[:].rearrange("p (j c) -> p j c", c=D),
                        in1=prm[:, 0:4].to_broadcast([TILE_E, 4, D]),
                        op=ALU.mult)
                    stm = ewp.tile([TILE_E, 512], f32, tag="e_stm")
                    nc.vector.tensor_tensor(
                        out=stm[:].rearrange("p (j c) -> p j c", c=D),
                        in0=iot[:].rearrange("p (j c) -> p j c", c=D),
                        in1=prm[:, 4:8].to_broadcast([TILE_E, 4, D]),
                        op=ALU.is_equal)
                    psA = psp.tile([D, D], f32, tag="ps1")
                    for j in range(4):
                        nc.tensor.matmul(
                            out=psA[:], lhsT=stm[:, j * D:(j + 1) * D],
                            rhs=mvg[:, j * D:(j + 1) * D],
                            start=(j == 0), stop=(j == 3))
                    sA = egp.tile([TILE_E, D], f32, tag="e_sA4")
                    nc.vector.tensor_scalar(
                        out=sA[:], in0=psA[:], scalar1=prm[:, 12:13],
                        scalar2=None, op0=ALU.mult)
                    nc.sync.dma_start(
                        out=stream_t[b * 128:(b + 1) * 128, :], in_=sA[:])

            def edge_phase_final(table_ap, idx_t, nb):
                for b in range(nb):
                    idxt = esmp.tile([TILE_E, 8], i32, tag="e_idx")
                    nc.sync.dma_start(out=idxt[:], in_=idx_t[b])
                    g = egp.tile([TILE_E, 8 * D], f32, tag="e_g")
                    nc.gpsimd.indirect_dma_start(
                        out=g[:], out_offset=None, in_=table_ap,
                        in_offset=bass.IndirectOffsetOnAxis(ap=idxt[:], axis=0),
                    )
                    tmp = ewp.tile([TILE_E, 512], f32, tag="e_tmp")
                    nc.vector.tensor_tensor(
                        out=tmp[:], in0=g[:, 0:512], in1=g[:, 512:1024],
                        op=ALU.mult)
                    dotp = ewp.tile([TILE_E, 4], f32, tag="e_dot")
                    nc.vector.reduce_sum(
                        out=dotp[:], in_=tmp[:].rearrange("p (j c) -> p j c", c=D),
                        axis=mybir.AxisListType.X)
                    nc.sync.dma_start(out=cosout[b], in_=dotp[:])

            # ---------------- node phase helpers --------------------------
            def w_apply(src_ap, wt):
                """Return PSUM tile holding src @ W.T (node-major in/out)."""
                psX = psp.tile([D, D], f32, tag="ps1")
                nc.tensor.transpose(out=psX[:], in_=src_ap, identity=ident[:])
                xT = npp.tile([D, D], f32, tag="n_xT")
                nc.vector.tensor_copy(out=xT[:], in_=psX[:])
                psH = psp2.tile([D, D], f32, tag="ps2")
                nc.tensor.matmul(out=psH[:], lhsT=xT[:], rhs=wt[:],
                                 start=True, stop=True)
                return psH

            def rinv_of(src_ap):
                """[128,1] tile: 1/max(||row||, eps)."""
                dmp = npp.tile([D, D], f32, tag="n_dmp")
                ssn = npp.tile([D, 1], f32, tag="n_ssn")
                nc.scalar.activation(out=dmp[:], in_=src_ap, func=AF.Square,
                                     accum_out=ssn[:])
                nc.scalar.activation(out=ssn[:], in_=ssn[:], func=AF.Sqrt)
                nc.vector.tensor_scalar(out=ssn[:], in0=ssn[:], scalar1=EPS,
                                        scalar2=None, op0=ALU.max)
                nc.vector.reciprocal(out=ssn[:], in_=ssn[:])
                return ssn

            def node_phase_ii(stream_t, posc, xprev_d, out_d, wkey, bkey,
                              then_w=None):
                """Finish an ii layer. xprev_d/out_d: DRAM [SLICE_P, D].
                If then_w: out rows are (x_next @ then_w.T) instead (h-table).
                """
                wt, bt = wts[wkey], wts[bkey]
                for q in range(NTILE_OWN // NODE_BLK):
                    gm = npp.tile([TILE_E, NODE_BLK * 256], f32, tag="n_gm")
                    nc.gpsimd.indirect_dma_start(
                        out=gm[:], out_offset=None, in_=stream_t[:, :],
                        in_offset=bass.IndirectOffsetOnAxis(
                            ap=posc[:, q * NODE_BLK:(q + 1) * NODE_BLK], axis=0),
                    )
                    for jj in range(NODE_BLK):
                        t = q * NODE_BLK + jj
                        xp = npp.tile([D, D], f32, tag="n_xp")
                        nc.sync.dma_start(
                            out=xp[:], in_=xprev_d[t * D:(t + 1) * D, :])
                        mk = maskc[:, t:t + 1]
                        ssn = rinv_of(xp[:])
                        # mean = rinv_d * (A @ W.T + B)
                        aslc = gm[:, jj * 256:jj * 256 + D]
                        bslc = gm[:, jj * 256 + D:(jj + 1) * 256]
                        psT = psp.tile([D, D], f32, tag="ps1")
                        nc.tensor.transpose(out=psT[:], in_=aslc, identity=ident[:])
                        aT = npp.tile([D, D], f32, tag="n_aT")
                        nc.vector.tensor_copy(out=aT[:], in_=psT[:])
                        psM = psp2.tile([D, D], f32, tag="ps2")
                        nc.tensor.matmul(out=psM[:], lhsT=aT[:], rhs=wt[:],
                                         start=True, stop=True)
                        mean = npp.tile([D, D], f32, tag="n_mean")
                        nc.vector.tensor_tensor(out=mean[:], in0=psM[:],
                                                in1=bslc, op=ALU.add)
                        nc.vector.tensor_scalar(out=mean[:], in0=mean[:],
                                                scalar1=ssn[:], scalar2=None,
                                                op0=ALU.mult)
                        # h = mask ? xprev @ W.T : xprev
                        psH = w_apply(xp[:], wt)
                        h = npp.tile([D, D], f32, tag="n_h")
                        nc.vector.tensor_copy(out=h[:], in_=xp[:])
                        nc.vector.copy_predicated(
                            out=h[:], mask=mk.to_broadcast([D, D]), data=psH[:])
                        # x_next = mask ? sigmoid(mean + h + b) : h
                        sg = npp.tile([D, D], f32, tag="n_sg")
                        nc.vector.tensor_tensor(out=sg[:], in0=mean[:], in1=h[:],
                                                op=ALU.add)
                        nc.vector.tensor_tensor(out=sg[:], in0=sg[:], in1=bt[:],
                                                op=ALU.add)
                        nc.scalar.activation(out=sg[:], in_=sg[:], func=AF.Sigmoid)
                        xn = npp.tile([D, D], f32, tag="n_xn")
                        nc.vector.tensor_copy(out=xn[:], in_=h[:])
                        nc.vector.copy_predicated(
                            out=xn[:], mask=mk.to_broadcast([D, D]), data=sg[:])
                        if then_w is not None:
                            psW = w_apply(xn[:], wts[then_w])
                            nc.vector.tensor_copy(out=xn[:], in_=psW[:])
                        nc.sync.dma_start(
                            out=out_d[t * D:(t + 1) * D, :], in_=xn[:])

            def node_phase_uiu(stream_t, posc, h_d, out_d, bkey, then_w=None,
                               then_norm=False):
                """u = sigmoid(mean + h + b); optional @W.T or normalize."""
                bt = wts[bkey]
                for q in range(NTILE_OWN // NODE_BLK):
                    gm = npp.tile([TILE_E, NODE_BLK * 128], f32, tag="n_gmu")
                    nc.gpsimd.indirect_dma_start(
                        out=gm[:], out_offset=None, in_=stream_t[:, :],
                        in_offset=bass.IndirectOffsetOnAxis(
                            ap=posc[:, q * NODE_BLK:(q + 1) * NODE_BLK], axis=0),
                    )
                    for jj in range(NODE_BLK):
                        t = q * NODE_BLK + jj
                        hp = npp.tile([D, D], f32, tag="n_xp")
                        nc.sync.dma_start(
                            out=hp[:], in_=h_d[t * D:(t + 1) * D, :])
                        sg = npp.tile([D, D], f32, tag="n_sg")
                        nc.vector.tensor_tensor(
                            out=sg[:], in0=gm[:, jj * D:(jj + 1) * D], in1=hp[:],
                            op=ALU.add)
                        nc.vector.tensor_tensor(out=sg[:], in0=sg[:], in1=bt[:],
                                                op=ALU.add)
                        nc.scalar.activation(out=sg[:], in_=sg[:], func=AF.Sigmoid)
                        if then_w is not None:
                            psW = w_apply(sg[:], wts[then_w])
                            nc.vector.tensor_copy(out=sg[:], in_=psW[:])
                        if then_norm:
                            ssn = rinv_of(sg[:])
                            nc.vector.tensor_scalar(
                                out=sg[:], in0=sg[:], scalar1=ssn[:],
                                scalar2=None, op0=ALU.mult)
                        nc.sync.dma_start(
                            out=out_d[t * D:(t + 1) * D, :], in_=sg[:])

            def allgather(ag_in, table):
                nc.gpsimd.collective_compute(
                    "AllGather", mybir.AluOpType.bypass,
                    ins=[ag_in.opt()], outs=[table.opt()],
                    replica_groups=[list(range(NCORES))],
                )

            # ======================= pipeline ==============================
            # E1: ii edges on input x
            edge_phase_ea(x_full[:], idx_ii, prm_ii, NBii, stream_i1)
            # P2: finish layer 1 -> x1 (agin0); AG -> tbl[0]
            node_phase_ii(stream_i1, posc_ii, x_own[:, :], agin[0], "w1", "b1")
            allgather(agin[0], tbl[0])
            # E2: ii edges on x1
            edge_phase_ea(tbl[0][:, :], idx_ii, prm_ii, NBii, stream_i2)
            # P3: finish layer 2 -> x2; h3 = x2 @ Wu.T -> agin1; AG
            node_phase_ii(stream_i2, posc_ii, agin[0], agin[1],
                          "w2", "b2", then_w="wu")
            allgather(agin[1], tbl[1])
            # E3: uiu edges on h3
            edge_phase_uiu(tbl[1][:, :], idx_uu, prm_uu, NBuu, stream_u3)
            # P4: u1 = sigmoid(mean + h3 + bu); h4 = u1 @ Wu.T -> agin2; AG
            node_phase_uiu(stream_u3, posc_uu, agin[1], agin[2], "bu",
                           then_w="wu")
            allgather(agin[2], tbl[2])
            # E4: uiu edges on h4
            edge_phase_uiu(tbl[2][:, :], idx_uu, prm_uu, NBuu, stream_u4)
            # P5: u2 = sigmoid(mean + h4 + bu); normalize -> agin3; AG
            node_phase_uiu(stream_u4, posc_uu, agin[2], agin[3], "bu",
                           then_norm=True)
            allgather(agin[3], tbl[3])
            # E5: final cosine on uiu edges
            edge_phase_final(tbl[3][:, :], idx_uu, NBuu)

    return nc


# ---------------------------------------------------------------------------
def _split_waits(nc, max_waits=1):
    """This walrus build rejects >1 semaphore wait per instruction; hoist
    excess waits onto same-engine NoOps inserted immediately before."""
    import concourse.mybir as mybir

    for fn in nc.m.functions:
        for blk in fn.blocks:
            out = []
            for inst in blk.instructions:
                si = inst.sync_info
                ow = list(si.on_wait) if si is not None and si.on_wait else []
                if len(ow) > max_waits:
                    extra, keep = ow[:-max_waits], ow[-max_waits:]
                    for i in range(0, len(extra), max_waits):
                        nop = mybir.InstNoOp(
                            name=nc.get_next_instruction_name(),
                            text_hint="wait_split", bass_nofuse=True)
                        nop.engine = inst.engine
                        nop.sync_info = mybir.SyncInfo(
                            on_wait=extra[i:i + max_waits], on_update=[])
                        nc.register_instruction(nop, overwrite=True)
                        out.append(nop)
                    si.on_wait = keep
                out.append(inst)
            blk.instructions = out


def _register_ntff_hook():
    try:
        from antenv.axon_hooks import (
            get_axon_ntff_profile_hook,
            set_axon_ntff_profile_hook,
        )
        if get_axon_ntff_profile_hook() is None:
            from trn_agent_boot.trn_boot import _ntff_profile_via_ctypes
            hook = _ntff_profile_via_ctypes("/opt/axon/libaxon_pjrt.so")
            if hook is not None:
                set_axon_ntff_profile_hook(hook)
    except Exception:
        pass


def kernel(**inputs):
    global LAST_EXEC_NS
    x = np.ascontiguousarray(np.asarray(inputs["x"], dtype=np.float32))
    eii = np.asarray(inputs["edge_index_ii"]).astype(np.int64)
    euu = np.asarray(inputs["edge_index_uiu"]).astype(np.int64)
    aii = np.asarray(inputs["edge_attr_ii"], dtype=np.float32)
    auu = np.asarray(inputs["edge_attr_uiu"], dtype=np.float32)
    w1 = np.asarray(inputs["W1_ii"], dtype=np.float32)
    w2 = np.asarray(inputs["W2_ii"], dtype=np.float32)
    wu = np.asarray(inputs["W_uiu"], dtype=np.float32)
    b1v = np.asarray(inputs["b1_ii"], dtype=np.float32)
    b2v = np.asarray(inputs["b2_ii"], dtype=np.float32)
    buv = np.asarray(inputs["b_uiu"], dtype=np.float32)
    mask = np.asarray(inputs["node_mask_item"]).astype(bool)

    gii, NBii = _prep_graph(eii[0], eii[1], aii, mask, mask, mask)
    guu, NBuu = _prep_graph(euu[0], euu[1], auu, None, None, None)

    # padded full-x table
    x_pad = np.zeros((NPAD, D), np.float32)
    for c in range(NCORES):
        x_pad[c * SLICE_P:c * SLICE_P + SLICE_R] = \
            x[c * SLICE_R:(c + 1) * SLICE_R]

    iota4 = np.tile(np.arange(128, dtype=np.float32)[None, :], (128, 4)) \
        .reshape(128, 512)
    iota4 = np.ascontiguousarray(
        np.broadcast_to(np.arange(128, dtype=np.float32)[None, :],
                        (128, 128)))
    iota4 = np.tile(iota4, (1, 4))

    nc = _build(NBii, NBuu)
    _split_waits(nc)
    _register_ntff_hook()

    from concourse.bass_utils import run_bass_kernel_spmd

    in_maps = []
    for c in range(NCORES):
        xo = np.zeros((SLICE_P, D), np.float32)
        xo[:SLICE_R] = x[c * SLICE_R:(c + 1) * SLICE_R]
        mo = np.zeros(SLICE_P, np.float32)
        mo[:SLICE_R] = mask[c * SLICE_R:(c + 1) * SLICE_R].astype(np.float32)
        maskt = np.ascontiguousarray(
            mo.reshape(NTILE_OWN, 128).T.astype(np.int8))
        in_maps.append({
            "x_full": x_pad,
            "x_own": xo,
            "w1t": np.ascontiguousarray(w1.T),
            "w2t": np.ascontiguousarray(w2.T),
            "wut": np.ascontiguousarray(wu.T),
            "b1": np.ascontiguousarray(np.tile(b1v, (128, 1))),
            "b2": np.ascontiguousarray(np.tile(b2v, (128, 1))),
            "bu": np.ascontiguousarray(np.tile(buv, (128, 1))),
            "iota4": np.ascontiguousarray(iota4),
            "maskt": maskt,
            "idx_ii": gii[c]["idx"],
            "prm_ii": gii[c]["prm"],
            "pos_ii": np.ascontiguousarray(gii[c]["posall"]),
            "idx_uu": guu[c]["idx"],
            "prm_uu": guu[c]["prm"],
            "pos_uu": np.ascontiguousarray(guu[c]["posall"]),
        })

    trace = bool(int(os.environ.get("KERNEL_TRACE", "0")))
    res = run_bass_kernel_spmd(nc, in_maps, core_ids=list(range(NCORES)),
                               trace=trace)
    LAST_EXEC_NS = res.exec_time_ns

    out = np.zeros(E, np.float32)
    for c in range(NCORES):
        cosv = res.results[c]["cosout"]            # [NBuu, 128, 4]
        orig = guu[c]["orig"]                      # [NBuu, 128, 4]
        sel = orig >= 0
        out[orig[sel]] = cosv[sel]
    return out


# revision 10
# speedup vs baseline: 1.0013x; 1.0013x over previous
"""Trainium2 Bass kernel for nn_BigraphModel (gnn_message_passing).

Strategy (8 NeuronCores, SPMD single NEFF):
  - Nodes are sharded into 8 equal contiguous ranges (12500 real + 44 pad rows
    per core so AllGather chunks are uniform 12544-row slices).
  - Edges are sharded by destination: every edge lands on the core that owns
    its dst node, so segment sums complete locally (no all-reduce).
  - Per layer, each core computes updated features for its owned nodes only;
    an AllGather replicates the per-layer gather table [100352, 128] to all
    cores. Layer 1 needs no AllGather (the full input x is already available).
  - Edge phase: big indirect-DMA row gathers (src/dst feature rows), per-edge
    cosine terms on DVE/ACT, and an in-tile segment-sum via a one-hot
    selection matmul on the PE (host precomputes per-edge slot ids; runs of a
    given dst never straddle a tile). Results stream to DRAM; the node phase
    gathers one stream row per owned node.
  - The linear layer W is applied after aggregation (linearity), so gather
    tables stay 128 channels wide.

Host-side numpy does only sharding/index prep: edge bucketing+sorting, slot
assignment, degree counts, padding, and final output reassembly.
"""

import os
import sys

import numpy as np

N, D, E, NCORES = 100000, 128, 600000, 8
SLICE_R = N // NCORES            # 12500 real nodes per core
SLICE_P = 12544                  # padded to multiple of 128
NPAD = SLICE_P * NCORES          # 100352 table rows
TILE_E = 128                     # edges per tile
TILE_S = 32                      # max slots (distinct dst) per tile
BLK = 4                          # tiles per superblock (4*32 = 128 psum slots)
NODE_BLK = 7                    # node tiles gathered per stream-gather call
NTILE_OWN = SLICE_P // 128       # 98
EPS = 1e-8

LAST_EXEC_NS = None
LAST_RESULTS = None


def _row_of_node(n):
    """Map node id -> padded table row."""
    return (n // SLICE_R) * SLICE_P + (n % SLICE_R)


def _prep_graph(src, dst, attr, dst_keep_mask, src_mask, split_by_src_mask):
    """Shard a graph's edges by dst owner; per core build tile/slot arrays.

    Returns (per_core list of dicts, NB) where every core has identical NB
    (superblock count), padded as needed.
    """
    cores = []
    owner = dst // SLICE_R
    cnt_all = np.bincount(dst, minlength=N)  # full in-degree (pre-filter)
    for c in range(NCORES):
        sel = owner == c
        if dst_keep_mask is not None:
            sel &= dst_keep_mask[dst]
        es, ed, ea = src[sel], dst[sel], attr[sel]
        eid = np.nonzero(sel)[0]
        order = np.argsort(ed, kind="stable")
        es, ed, ea, eid = es[order], ed[order], ea[order], eid[order]
        # run boundaries (consecutive equal dst)
        if len(ed):
            bnd = np.nonzero(np.diff(ed))[0] + 1
            starts = np.concatenate(([0], bnd))
            ends = np.concatenate((bnd, [len(ed)]))
        else:
            starts = ends = np.zeros(0, np.int64)
        run_len = ends - starts
        if len(run_len) and run_len.max() > TILE_E:
            raise ValueError("in-degree > 128 unsupported by this kernel")
        # greedy tile packing: <=128 edges, <=32 runs per tile
        tiles = []  # list of list of run indices
        cur, ce, cr = [], 0, 0
        for r in range(len(starts)):
            L = int(run_len[r])
            if ce + L > TILE_E or cr + 1 > TILE_S:
                tiles.append(cur)
                cur, ce, cr = [], 0, 0
            cur.append(r)
            ce += L
            cr += 1
        if cur:
            tiles.append(cur)
        cores.append(
            dict(es=es, ed=ed, ea=ea, eid=eid, starts=starts, ends=ends,
                 tiles=tiles, cnt=cnt_all)
        )
    nt_max = max(len(c["tiles"]) for c in cores)
    nb = max(1, -(-nt_max // BLK))
    nt_pad = nb * BLK
    out = []
    for c in range(NCORES):
        g = cores[c]
        tiles = g["tiles"]
        idx8 = np.zeros((nt_pad, TILE_E, 2), np.int32)      # [t,p,(src,dst)]
        attr_a = np.zeros((nt_pad, TILE_E), np.float32)
        sid_m = np.full((nt_pad, TILE_E), -1.0, np.float32)
        sid_u = np.full((nt_pad, TILE_E), -1.0, np.float32)
        rcnt = np.zeros((nt_pad, TILE_S), np.float32)
        pos = np.full(SLICE_P, nt_pad * TILE_S, np.int64)   # zero-row default
        orig = np.full((nt_pad, TILE_E), -1, np.int64)
        for t, runs in enumerate(tiles):
            p = 0
            for s, r in enumerate(runs):
                a, b = int(g["starts"][r]), int(g["ends"][r])
                L = b - a
                d_node = int(g["ed"][a])
                bias = (t % BLK) * TILE_S
                idx8[t, p:p + L, 0] = _row_of_node(g["es"][a:b])
                idx8[t, p:p + L, 1] = _row_of_node(np.int64(d_node))
                attr_a[t, p:p + L] = g["ea"][a:b]
                if split_by_src_mask is not None:
                    sm = split_by_src_mask[g["es"][a:b]]
                    sid_m[t, p:p + L] = np.where(sm, float(s + bias), -1.0)
                    sid_u[t, p:p + L] = np.where(sm, -1.0, float(s + bias))
                else:
                    sid_m[t, p:p + L] = float(s + bias)
                rcnt[t, s] = 1.0 / max(int(g["cnt"][d_node]), 1)
                # stream row for this dst: block*128 + (t%4)*32 + s
                pos[d_node % SLICE_R] = (t // BLK) * 128 + bias + s
                orig[t, p:p + L] = g["eid"][a:b]
                p += L
        # reshape into superblock layout [NB, 128, cols]
        i8 = idx8.reshape(nb, BLK, TILE_E, 2)
        idx = np.zeros((nb, TILE_E, 8), np.int32)
        for j in range(BLK):
            idx[:, :, j] = i8[:, j, :, 0]
            idx[:, :, 4 + j] = i8[:, j, :, 1]
        prm = np.zeros((nb, TILE_E, 13), np.float32)
        at = attr_a.reshape(nb, BLK, TILE_E)
        sm_ = sid_m.reshape(nb, BLK, TILE_E)
        su_ = sid_u.reshape(nb, BLK, TILE_E)
        for j in range(BLK):
            prm[:, :, j] = at[:, j]
            prm[:, :, 4 + j] = sm_[:, j]
            prm[:, :, 8 + j] = su_[:, j]
        prm[:, :, 12] = rcnt.reshape(nb, 128)
        posall = pos.reshape(NTILE_OWN, 128).T.astype(np.int32)  # [128, 98]
        orig_b = np.zeros((nb, TILE_E, BLK), np.int64)
        ob = orig.reshape(nb, BLK, TILE_E)
        for j in range(BLK):
            orig_b[:, :, j] = ob[:, j]
        out.append(dict(idx=idx, prm=prm, posall=posall, orig=orig_b))
    return out, nb


def _build(NBii, NBuu):
    import concourse.bass as bass
    import concourse.mybir as mybir
    import concourse.tile as tile
    from concourse import library_config
    from concourse.masks import make_identity
    from concourse.tile_rust import add_dep_helper

    f32 = mybir.dt.float32
    i32 = mybir.dt.int32
    AF = mybir.ActivationFunctionType
    ALU = mybir.AluOpType

    nc = bass.Bass()

    # ---- external inputs -------------------------------------------------
    x_full = nc.dram_tensor("x_full", [NPAD, D], f32, kind="ExternalInput")
    x_own = nc.dram_tensor("x_own", [SLICE_P, D], f32, kind="ExternalInput")
    w1t = nc.dram_tensor("w1t", [D, D], f32, kind="ExternalInput")
    w2t = nc.dram_tensor("w2t", [D, D], f32, kind="ExternalInput")
    wut = nc.dram_tensor("wut", [D, D], f32, kind="ExternalInput")
    b1 = nc.dram_tensor("b1", [D, D], f32, kind="ExternalInput")
    b2 = nc.dram_tensor("b2", [D, D], f32, kind="ExternalInput")
    bu = nc.dram_tensor("bu", [D, D], f32, kind="ExternalInput")
    iota4 = nc.dram_tensor("iota4", [D, 512], f32, kind="ExternalInput")
    maskt = nc.dram_tensor("maskt", [D, NTILE_OWN], mybir.dt.int8, kind="ExternalInput")
    idx_ii = nc.dram_tensor("idx_ii", [NBii, TILE_E, 8], i32, kind="ExternalInput")
    prm_ii = nc.dram_tensor("prm_ii", [NBii, TILE_E, 13], f32, kind="ExternalInput")
    pos_ii = nc.dram_tensor("pos_ii", [D, NTILE_OWN], i32, kind="ExternalInput")
    idx_uu = nc.dram_tensor("idx_uu", [NBuu, TILE_E, 8], i32, kind="ExternalInput")
    prm_uu = nc.dram_tensor("prm_uu", [NBuu, TILE_E, 13], f32, kind="ExternalInput")
    pos_uu = nc.dram_tensor("pos_uu", [D, NTILE_OWN], i32, kind="ExternalInput")
    cosout = nc.dram_tensor("cosout", [NBuu, TILE_E, 4], f32, kind="ExternalOutput")
    dbg = [nc.dram_tensor(f"dbg{k}", [SLICE_P, D], f32, kind="ExternalOutput")
           for k in range(4)] if os.environ.get("KERNEL_DEBUG") else None

    NSii = NBii * 128 + 128   # stream rows (+128 pad incl. zero row)
    NSuu = NBuu * 128 + 128
    ZRii = NBii * 128
    ZRuu = NBuu * 128

    with tile.TileContext(nc) as tc:
        with (
            tc.tile_pool(name="dram", bufs=1, space="DRAM") as dram,
            tc.tile_pool(name="const", bufs=1) as constp,
            tc.tile_pool(name="eg", bufs=3) as egp,
            tc.tile_pool(name="esm", bufs=3) as esmp,
            tc.tile_pool(name="ework", bufs=3) as ewp,
            tc.tile_pool(name="npool", bufs=3) as npp,
            tc.tile_pool(name="psum", bufs=2, space="PSUM") as psp,
            tc.tile_pool(name="psum2", bufs=2, space="PSUM") as psp2,
        ):
            # DRAM intermediates
            stream_i1 = dram.tile([NSii, 256], f32, tag="st_i1")
            stream_i2 = dram.tile([NSii, 256], f32, tag="st_i2")
            stream_u3 = dram.tile([NSuu, 128], f32, tag="st_u3")
            stream_u4 = dram.tile([NSuu, 128], f32, tag="st_u4")
            agin = [dram.tile([SLICE_P, D], f32, tag=f"agin{k}", name=f"agin{k}") for k in range(4)]
            tbl = [dram.tile([NPAD, D], f32, tag=f"tbl{k}", name=f"tbl{k}") for k in range(4)]

            # constants
            ident = constp.tile([D, D], f32, tag="ident")
            make_identity(nc, ident[:])
            iot = constp.tile([D, 512], f32, tag="iot")
            nc.sync.dma_start(out=iot[:], in_=iota4[:])
            wts = {}
            for nm, t in (("w1", w1t), ("w2", w2t), ("wu", wut),
                          ("b1", b1), ("b2", b2), ("bu", bu)):
                wt = constp.tile([D, D], f32, tag=f"c_{nm}", name=f"c_{nm}")
                nc.sync.dma_start(out=wt[:], in_=t[:])
                wts[nm] = wt
            maskc = constp.tile([D, NTILE_OWN], mybir.dt.int8, tag="maskc")
            nc.sync.dma_start(out=maskc[:], in_=maskt[:])
            posc_ii = constp.tile([D, NTILE_OWN], i32, tag="posc_ii")
            nc.sync.dma_start(out=posc_ii[:], in_=pos_ii[:])
            posc_uu = constp.tile([D, NTILE_OWN], i32, tag="posc_uu")
            nc.sync.dma_start(out=posc_uu[:], in_=pos_uu[:])
            zrow = constp.tile([D, 256], f32, tag="zrow")
            nc.vector.memset(zrow[:], 0.0)
            # zero the pad tail of every stream (gathered rows must be finite)
            zw1 = nc.sync.dma_start(out=stream_i1[ZRii:ZRii + 128, :],
                                    in_=zrow[:, :256])
            zw2 = nc.sync.dma_start(out=stream_i2[ZRii:ZRii + 128, :],
                                    in_=zrow[:, :256])
            zw3 = nc.sync.dma_start(out=stream_u3[ZRuu:ZRuu + 128, :],
                                    in_=zrow[:, :128])
            zw4 = nc.sync.dma_start(out=stream_u4[ZRuu:ZRuu + 128, :],
                                    in_=zrow[:, :128])


            # ---------------- edge phase helpers --------------------------
            def edge_phase_ea(table_ap, idx_t, prm_t, nb, stream_t,
                              dep_src=None):
                writes = []
                for b in range(nb):
                    idxt = esmp.tile([TILE_E, 8], i32, tag="e_idx")
                    nc.sync.dma_start(out=idxt[:], in_=idx_t[b])
                    prm = esmp.tile([TILE_E, 13], f32, tag="e_prm")
                    nc.sync.dma_start(out=prm[:], in_=prm_t[b])
                    g = egp.tile([TILE_E, 8 * D], f32, tag="e_g")
                    gi = nc.gpsimd.indirect_dma_start(
                        out=g[:], out_offset=None, in_=table_ap,
                        in_offset=bass.IndirectOffsetOnAxis(ap=idxt[:], axis=0),
                    )
                    if dep_src is not None:
                        add_dep_helper(gi.ins, dep_src.ins, True, "gather waits on AG")
                    gs = g[:, 0:512].rearrange("p (j c) -> p j c", c=D)
                    gd = g[:, 512:1024].rearrange("p (j c) -> p j c", c=D)
                    # per-edge dot(x_s, x_d)
                    tmp = ewp.tile([TILE_E, 512], f32, tag="e_tmp")
                    nc.vector.tensor_tensor(
                        out=tmp[:], in0=g[:, 0:512], in1=g[:, 512:1024],
                        op=ALU.mult)
                    dotp = ewp.tile([TILE_E, 4], f32, tag="e_dot")
                    nc.vector.reduce_sum(
                        out=dotp[:], in_=tmp[:].rearrange("p (j c) -> p j c", c=D),
                        axis=mybir.AxisListType.X)
                    # per-edge ||x_s||^2 via ACT square+accum
                    ssq = ewp.tile([TILE_E, 4], f32, tag="e_ssq")
                    dump = ewp.tile([TILE_E, D], f32, tag="e_dump")
                    for j in range(4):
                        nc.scalar.activation(
                            out=dump[:], in_=gs[:, j, :], func=AF.Square,
                            accum_out=ssq[:, j:j + 1])
                    nrm = ewp.tile([TILE_E, 4], f32, tag="e_nrm")
                    nc.scalar.activation(out=nrm[:], in_=ssq[:], func=AF.Sqrt)
                    nc.vector.tensor_scalar(
                        out=nrm[:], in0=nrm[:], scalar1=EPS, scalar2=None,
                        op0=ALU.max)
                    nc.vector.reciprocal(out=nrm[:], in_=nrm[:])
                    beta = ewp.tile([TILE_E, 4], f32, tag="e_beta")
                    nc.vector.tensor_tensor(
                        out=beta[:], in0=dotp[:], in1=prm[:, 0:4], op=ALU.mult)
                    nc.vector.tensor_tensor(
                        out=beta[:], in0=beta[:], in1=nrm[:], op=ALU.mult)
                    # messages beta * x_s
                    mvg = ewp.tile([TILE_E, 512], f32, tag="e_mvg")
                    nc.vector.tensor_tensor(
                        out=mvg[:].rearrange("p (j c) -> p j c", c=D),
                        in0=gs, in1=beta[:].to_broadcast([TILE_E, 4, D]),
                        op=ALU.mult)
                    # selection matrices (masked / unmasked src)
                    stm = ewp.tile([TILE_E, 512], f32, tag="e_stm")
                    nc.vector.tensor_tensor(
                        out=stm[:].rearrange("p (j c) -> p j c", c=D),
                        in0=iot[:].rearrange("p (j c) -> p j c", c=D),
                        in1=prm[:, 4:8].to_broadcast([TILE_E, 4, D]),
                        op=ALU.is_equal)
                    stu = ewp.tile([TILE_E, 512], f32, tag="e_stu")
                    nc.vector.tensor_tensor(
                        out=stu[:].rearrange("p (j c) -> p j c", c=D),
                        in0=iot[:].rearrange("p (j c) -> p j c", c=D),
                        in1=prm[:, 8:12].to_broadcast([TILE_E, 4, D]),
                        op=ALU.is_equal)
                    psA = psp.tile([D, D], f32, tag="ps1")
                    psB = psp2.tile([D, D], f32, tag="ps2")
                    for j in range(4):
                        nc.tensor.matmul(
                            out=psA[:], lhsT=stm[:, j * D:(j + 1) * D],
                            rhs=mvg[:, j * D:(j + 1) * D],
                            start=(j == 0), stop=(j == 3))
                    for j in range(4):
                        nc.tensor.matmul(
                            out=psB[:], lhsT=stu[:, j * D:(j + 1) * D],
                            rhs=mvg[:, j * D:(j + 1) * D],
                            start=(j == 0), stop=(j == 3))
                    sA = egp.tile([TILE_E, 256], f32, tag="e_sA")
                    nc.vector.tensor_scalar(
                        out=sA[:, 0:D], in0=psA[:], scalar1=prm[:, 12:13],
                        scalar2=None, op0=ALU.mult)
                    nc.vector.tensor_scalar(
                        out=sA[:, D:256], in0=psB[:], scalar1=prm[:, 12:13],
                        scalar2=None, op0=ALU.mult)
                    writes.append(nc.sync.dma_start(
                        out=stream_t[b * 128:(b + 1) * 128, :], in_=sA[:]))
                return writes

            def edge_phase_uiu(table_ap, idx_t, prm_t, nb, stream_t,
                               dep_src=None):
                writes = []
                for b in range(nb):
                    idxt = esmp.tile([TILE_E, 4], i32, tag="e_idx4")
                    nc.sync.dma_start(out=idxt[:], in_=idx_t[b, :, 0:4])
                    prm = esmp.tile([TILE_E, 13], f32, tag="e_prm")
                    nc.sync.dma_start(out=prm[:], in_=prm_t[b])
                    g = egp.tile([TILE_E, 4 * D], f32, tag="e_g4")
                    gi = nc.gpsimd.indirect_dma_start(
                        out=g[:], out_offset=None, in_=table_ap,
                        in_offset=bass.IndirectOffsetOnAxis(ap=idxt[:], axis=0),
                    )
                    if dep_src is not None:
                        add_dep_helper(gi.ins, dep_src.ins, True, "gather waits on AG")
                    mvg = ewp.tile([TILE_E, 512], f32, tag="e_mvg")
                    nc.vector.tensor_tensor(
                        out=mvg[:].rearrange("p (j c) -> p j c", c=D),
                        in0=g[:].rearrange("p (j c) -> p j c", c=D),
                        in1=prm[:, 0:4].to_broadcast([TILE_E, 4, D]),
                        op=ALU.mult)
                    stm = ewp.tile([TILE_E, 512], f32, tag="e_stm")
                    nc.vector.tensor_tensor(
                        out=stm[:].rearrange("p (j c) -> p j c", c=D),
                        in0=iot[:].rearrange("p (j c) -> p j c", c=D),
                        in1=prm[:, 4:8].to_broadcast([TILE_E, 4, D]),
                        op=ALU.is_equal)
                    psA = psp.tile([D, D], f32, tag="ps1")
                    for j in range(4):
                        nc.tensor.matmul(
                            out=psA[:], lhsT=stm[:, j * D:(j + 1) * D],
                            rhs=mvg[:, j * D:(j + 1) * D],
                            start=(j == 0), stop=(j == 3))
                    sA = egp.tile([TILE_E, D], f32, tag="e_sA4")
                    nc.vector.tensor_scalar(
                        out=sA[:], in0=psA[:], scalar1=prm[:, 12:13],
                        scalar2=None, op0=ALU.mult)
                    writes.append(nc.sync.dma_start(
                        out=stream_t[b * 128:(b + 1) * 128, :], in_=sA[:]))
                return writes

            def edge_phase_final(table_ap, idx_t, nb, dep_src=None):
                for b in range(nb):
                    idxt = esmp.tile([TILE_E, 8], i32, tag="e_idx")
                    nc.sync.dma_start(out=idxt[:], in_=idx_t[b])
                    g = egp.tile([TILE_E, 8 * D], f32, tag="e_g")
                    gi = nc.gpsimd.indirect_dma_start(
                        out=g[:], out_offset=None, in_=table_ap,
                        in_offset=bass.IndirectOffsetOnAxis(ap=idxt[:], axis=0),
                    )
                    if dep_src is not None:
                        add_dep_helper(gi.ins, dep_src.ins, True, "gather waits on AG")
                    tmp = ewp.tile([TILE_E, 512], f32, tag="e_tmp")
                    nc.vector.tensor_tensor(
                        out=tmp[:], in0=g[:, 0:512], in1=g[:, 512:1024],
                        op=ALU.mult)
                    dotp = ewp.tile([TILE_E, 4], f32, tag="e_dot")
                    nc.vector.reduce_sum(
                        out=dotp[:], in_=tmp[:].rearrange("p (j c) -> p j c", c=D),
                        axis=mybir.AxisListType.X)
                    nc.sync.dma_start(out=cosout[b], in_=dotp[:])

            # ---------------- node phase helpers --------------------------
            def w_apply(src_ap, wt):
                """Return PSUM tile holding src @ W.T (node-major in/out)."""
                psX = psp.tile([D, D], f32, tag="ps1")
                nc.tensor.transpose(out=psX[:], in_=src_ap, identity=ident[:])
                xT = npp.tile([D, D], f32, tag="n_xT")
                nc.vector.tensor_copy(out=xT[:], in_=psX[:])
                psH = psp2.tile([D, D], f32, tag="ps2")
                nc.tensor.matmul(out=psH[:], lhsT=xT[:], rhs=wt[:],
                                 start=True, stop=True)
                return psH

            def rinv_of(src_ap):
                """[128,1] tile: 1/max(||row||, eps)."""
                dmp = npp.tile([D, D], f32, tag="n_dmp")
                ssn = npp.tile([D, 1], f32, tag="n_ssn")
                nc.scalar.activation(out=dmp[:], in_=src_ap, func=AF.Square,
                                     accum_out=ssn[:])
                nc.scalar.activation(out=ssn[:], in_=ssn[:], func=AF.Sqrt)
                nc.vector.tensor_scalar(out=ssn[:], in0=ssn[:], scalar1=EPS,
                                        scalar2=None, op0=ALU.max)
                nc.vector.reciprocal(out=ssn[:], in_=ssn[:])
                return ssn

            def node_phase_ii(stream_t, posc, xprev_d, out_d, wkey, bkey,
                              then_w=None, stream_deps=()):
                """Finish an ii layer. xprev_d/out_d: DRAM [SLICE_P, D].
                If then_w: out rows are (x_next @ then_w.T) instead (h-table).
                """
                wt, bt = wts[wkey], wts[bkey]
                outw = []
                for q in range(NTILE_OWN // NODE_BLK):
                    gm = npp.tile([TILE_E, NODE_BLK * 256], f32, tag="n_gm")
                    gmi = nc.gpsimd.indirect_dma_start(
                        out=gm[:], out_offset=None, in_=stream_t[:, :],
                        in_offset=bass.IndirectOffsetOnAxis(
                            ap=posc[:, q * NODE_BLK:(q + 1) * NODE_BLK], axis=0),
                    )
                    for w in stream_deps:
                        add_dep_helper(gmi.ins, w.ins, True, "gather waits on stream write")
                    for jj in range(NODE_BLK):
                        t = q * NODE_BLK + jj
                        xp = npp.tile([D, D], f32, tag="n_xp")
                        nc.sync.dma_start(
                            out=xp[:], in_=xprev_d[t * D:(t + 1) * D, :])
                        mk = maskc[:, t:t + 1]
                        ssn = rinv_of(xp[:])
                        # mean = rinv_d * (A @ W.T + B)
                        aslc = gm[:, jj * 256:jj * 256 + D]
                        bslc = gm[:, jj * 256 + D:(jj + 1) * 256]
                        psT = psp.tile([D, D], f32, tag="ps1")
                        nc.tensor.transpose(out=psT[:], in_=aslc, identity=ident[:])
                        aT = npp.tile([D, D], f32, tag="n_aT")
                        nc.vector.tensor_copy(out=aT[:], in_=psT[:])
                        psM = psp2.tile([D, D], f32, tag="ps2")
                        nc.tensor.matmul(out=psM[:], lhsT=aT[:], rhs=wt[:],
                                         start=True, stop=True)
                        mean = npp.tile([D, D], f32, tag="n_mean")
                        nc.vector.tensor_tensor(out=mean[:], in0=psM[:],
                                                in1=bslc, op=ALU.add)
                        nc.vector.tensor_scalar(out=mean[:], in0=mean[:],
                                                scalar1=ssn[:], scalar2=None,
                                                op0=ALU.mult)
                        # h = mask ? xprev @ W.T : xprev
                        psH = w_apply(xp[:], wt)
                        h = npp.tile([D, D], f32, tag="n_h")
                        nc.vector.tensor_copy(out=h[:], in_=xp[:])
                        nc.vector.copy_predicated(
                            out=h[:], mask=mk.to_broadcast([D, D]), data=psH[:])
                        # x_next = mask ? sigmoid(mean + h + b) : h
                        sg = npp.tile([D, D], f32, tag="n_sg")
                        nc.vector.tensor_tensor(out=sg[:], in0=mean[:], in1=h[:],
                                                op=ALU.add)
                        nc.vector.tensor_tensor(out=sg[:], in0=sg[:], in1=bt[:],
                                                op=ALU.add)
                        nc.scalar.activation(out=sg[:], in_=sg[:], func=AF.Sigmoid)
                        xn = npp.tile([D, D], f32, tag="n_xn")
                        nc.vector.tensor_copy(out=xn[:], in_=h[:])
                        nc.vector.copy_predicated(
                            out=xn[:], mask=mk.to_broadcast([D, D]), data=sg[:])
                        if then_w is not None:
                            psW = w_apply(xn[:], wts[then_w])
                            nc.vector.tensor_copy(out=xn[:], in_=psW[:])
                        outw.append(nc.sync.dma_start(
                            out=out_d[t * D:(t + 1) * D, :], in_=xn[:]))
                return outw

            def node_phase_uiu(stream_t, posc, h_d, out_d, bkey, then_w=None,
                               then_norm=False, stream_deps=()):
                """u = sigmoid(mean + h + b); optional @W.T or normalize."""
                bt = wts[bkey]
                outw = []
                for q in range(NTILE_OWN // NODE_BLK):
                    gm = npp.tile([TILE_E, NODE_BLK * 128], f32, tag="n_gmu")
                    gmi = nc.gpsimd.indirect_dma_start(
                        out=gm[:], out_offset=None, in_=stream_t[:, :],
                        in_offset=bass.IndirectOffsetOnAxis(
                            ap=posc[:, q * NODE_BLK:(q + 1) * NODE_BLK], axis=0),
                    )
                    for w in stream_deps:
                        add_dep_helper(gmi.ins, w.ins, True, "gather waits on stream write")
                    for jj in range(NODE_BLK):
                        t = q * NODE_BLK + jj
                        hp = npp.tile([D, D], f32, tag="n_xp")
                        nc.sync.dma_start(
                            out=hp[:], in_=h_d[t * D:(t + 1) * D, :])
                        sg = npp.tile([D, D], f32, tag="n_sg")
                        nc.vector.tensor_tensor(
                            out=sg[:], in0=gm[:, jj * D:(jj + 1) * D], in1=hp[:],
                            op=ALU.add)
                        nc.vector.tensor_tensor(out=sg[:], in0=sg[:], in1=bt[:],
                                                op=ALU.add)
                        nc.scalar.activation(out=sg[:], in_=sg[:], func=AF.Sigmoid)
                        if then_w is not None:
                            psW = w_apply(sg[:], wts[then_w])
                            nc.vector.tensor_copy(out=sg[:], in_=psW[:])
                        if then_norm:
                            ssn = rinv_of(sg[:])
                            nc.vector.tensor_scalar(
                                out=sg[:], in0=sg[:], scalar1=ssn[:],
                                scalar2=None, op0=ALU.mult)
                        outw.append(nc.sync.dma_start(
                            out=out_d[t * D:(t + 1) * D, :], in_=sg[:]))
                return outw

            def allgather(ag_in, table, in_deps=()):
                agi = nc.gpsimd.collective_compute(
                    "AllGather", mybir.AluOpType.bypass,
                    ins=[ag_in.opt()], outs=[table.opt()],
                    replica_groups=[list(range(NCORES))],
                )
                for w in in_deps:
                    add_dep_helper(agi.ins, w.ins, True, "AG waits on agin write")
                return agi

            # ======================= pipeline ==============================
            w1l = edge_phase_ea(x_full[:], idx_ii, prm_ii, NBii, stream_i1)
            a0w = node_phase_ii(stream_i1, posc_ii, x_own[:, :], agin[0],
                                "w1", "b1", stream_deps=w1l + [zw1])
            ag0 = allgather(agin[0], tbl[0], in_deps=a0w)
            w2l = edge_phase_ea(tbl[0][:, :], idx_ii, prm_ii, NBii, stream_i2,
                                dep_src=ag0)
            a1w = node_phase_ii(stream_i2, posc_ii, agin[0], agin[1],
                                "w2", "b2", then_w="wu",
                                stream_deps=w2l + [zw2])
            ag1 = allgather(agin[1], tbl[1], in_deps=a1w)
            w3l = edge_phase_uiu(tbl[1][:, :], idx_uu, prm_uu, NBuu, stream_u3,
                                 dep_src=ag1)
            a2w = node_phase_uiu(stream_u3, posc_uu, agin[1], agin[2], "bu",
                                 then_w="wu", stream_deps=w3l + [zw3])
            ag2 = allgather(agin[2], tbl[2], in_deps=a2w)
            w4l = edge_phase_uiu(tbl[2][:, :], idx_uu, prm_uu, NBuu, stream_u4,
                                 dep_src=ag2)
            a3w = node_phase_uiu(stream_u4, posc_uu, agin[2], agin[3], "bu",
                                 then_norm=True, stream_deps=w4l + [zw4])
            ag3 = allgather(agin[3], tbl[3], in_deps=a3w)
            edge_phase_final(tbl[3][:, :], idx_uu, NBuu, dep_src=ag3)
            if dbg is not None:
                for k in range(4):
                    nc.sync.dma_start(out=dbg[k][:, :], in_=agin[k][:, :])

    return nc


# ---------------------------------------------------------------------------
def _split_waits(nc, max_waits=1):
    """This walrus build rejects >1 semaphore wait per instruction; hoist
    excess waits onto same-engine NoOps inserted immediately before."""
    import concourse.mybir as mybir

    for fn in nc.m.functions:
        for blk in fn.blocks:
            out = []
            for inst in blk.instructions:
                si = inst.sync_info
                ow = list(si.on_wait) if si is not None and si.on_wait else []
                if len(ow) > max_waits:
                    extra, keep = ow[:-max_waits], ow[-max_waits:]
                    for i in range(0, len(extra), max_waits):
                        nop = mybir.InstNoOp(
                            name=nc.get_next_instruction_name(),
                            text_hint="wait_split", bass_nofuse=True)
                        nop.engine = inst.engine
                        nop.sync_info = mybir.SyncInfo(
                            on_wait=extra[i:i + max_waits], on_update=[])
                        nc.register_instruction(nop, overwrite=True)
                        out.append(nop)
                    si.on_wait = keep
                out.append(inst)
            blk.instructions = out


def _register_ntff_hook():
    try:
        from antenv.axon_hooks import (
            get_axon_ntff_profile_hook,
            set_axon_ntff_profile_hook,
        )
        if get_axon_ntff_profile_hook() is None:
            from trn_agent_boot.trn_boot import _ntff_profile_via_ctypes
            hook = _ntff_profile_via_ctypes("/opt/axon/libaxon_pjrt.so")
            if hook is not None:
                set_axon_ntff_profile_hook(hook)
    except Exception:
        pass


def kernel(**inputs):
    global LAST_EXEC_NS, LAST_RESULTS
    x = np.ascontiguousarray(np.asarray(inputs["x"], dtype=np.float32))
    eii = np.asarray(inputs["edge_index_ii"]).astype(np.int64)
    euu = np.asarray(inputs["edge_index_uiu"]).astype(np.int64)
    aii = np.asarray(inputs["edge_attr_ii"], dtype=np.float32)
    auu = np.asarray(inputs["edge_attr_uiu"], dtype=np.float32)
    w1 = np.asarray(inputs["W1_ii"], dtype=np.float32)
    w2 = np.asarray(inputs["W2_ii"], dtype=np.float32)
    wu = np.asarray(inputs["W_uiu"], dtype=np.float32)
    b1v = np.asarray(inputs["b1_ii"], dtype=np.float32)
    b2v = np.asarray(inputs["b2_ii"], dtype=np.float32)
    buv = np.asarray(inputs["b_uiu"], dtype=np.float32)
    mask = np.asarray(inputs["node_mask_item"]).astype(bool)

    gii, NBii = _prep_graph(eii[0], eii[1], aii, mask, mask, mask)
    guu, NBuu = _prep_graph(euu[0], euu[1], auu, None, None, None)

    # padded full-x table
    x_pad = np.zeros((NPAD, D), np.float32)
    for c in range(NCORES):
        x_pad[c * SLICE_P:c * SLICE_P + SLICE_R] = \
            x[c * SLICE_R:(c + 1) * SLICE_R]

    iota4 = np.tile(np.arange(128, dtype=np.float32)[None, :], (128, 4)) \
        .reshape(128, 512)
    iota4 = np.ascontiguousarray(
        np.broadcast_to(np.arange(128, dtype=np.float32)[None, :],
                        (128, 128)))
    iota4 = np.tile(iota4, (1, 4))

    nc = _build(NBii, NBuu)
    _split_waits(nc)
    _register_ntff_hook()

    from concourse.bass_utils import run_bass_kernel_spmd

    in_maps = []
    for c in range(NCORES):
        xo = np.zeros((SLICE_P, D), np.float32)
        xo[:SLICE_R] = x[c * SLICE_R:(c + 1) * SLICE_R]
        mo = np.zeros(SLICE_P, np.float32)
        mo[:SLICE_R] = mask[c * SLICE_R:(c + 1) * SLICE_R].astype(np.float32)
        maskt = np.ascontiguousarray(
            mo.reshape(NTILE_OWN, 128).T.astype(np.int8))
        in_maps.append({
            "x_full": x_pad,
            "x_own": xo,
            "w1t": np.ascontiguousarray(w1.T),
            "w2t": np.ascontiguousarray(w2.T),
            "wut": np.ascontiguousarray(wu.T),
            "b1": np.ascontiguousarray(np.tile(b1v, (128, 1))),
            "b2": np.ascontiguousarray(np.tile(b2v, (128, 1))),
            "bu": np.ascontiguousarray(np.tile(buv, (128, 1))),
            "iota4": np.ascontiguousarray(iota4),
            "maskt": maskt,
            "idx_ii": gii[c]["idx"],
            "prm_ii": gii[c]["prm"],
            "pos_ii": np.ascontiguousarray(gii[c]["posall"]),
            "idx_uu": guu[c]["idx"],
            "prm_uu": guu[c]["prm"],
            "pos_uu": np.ascontiguousarray(guu[c]["posall"]),
        })

    trace = bool(int(os.environ.get("KERNEL_TRACE", "0")))
    res = run_bass_kernel_spmd(nc, in_maps, core_ids=list(range(NCORES)),
                               trace=trace)
    LAST_EXEC_NS = res.exec_time_ns
    LAST_RESULTS = res.results

    out = np.zeros(E, np.float32)
    for c in range(NCORES):
        cosv = res.results[c]["cosout"]            # [NBuu, 128, 4]
        orig = guu[c]["orig"]                      # [NBuu, 128, 4]
        sel = orig >= 0
        out[orig[sel]] = cosv[sel]
    return out
